# revision 30
# baseline (speedup 1.0000x reference)
"""Trainium2 Bass kernel for nn_BERT_936302870555 (v2).

Sharding: 8 cores; core c -> (batch b = c%4, group g = c//4). Each core's
1200-token row is two 600-token halves at different layer chains:
  g=0: half0 = stack0 chain (9 layers), half1 = s1 chain (5 layers)
  g=1: half0 = s2 chain (5 layers),    half1 = s3 chain (5 layers)
Program schedule (same for all cores): half0 heads H0=[1,2,3,4,6,9,12,18,36],
half1 heads H1=[1,2,3,4,6,9,12]; chains embed as subsequences, missing
(slot,half) entries run with zero weights (residual pass-through).
All matmuls run in fp32 (accuracy gate needs ~1e-5 abs).
Second launch: SE-gated concat -> conv1d(k=3) -> BN -> ReLU epilogue.
"""

import numpy as np

D = 36
L = 600
B = 4
T = 2 * L
NSLOT = 9
H0 = [1, 2, 3, 4, 6, 9, 12, 18, 36]
H1 = [1, 2, 3, 4, 6, 9, 12]
HEADS = [1, 2, 3, 4, 6, 9, 12, 18, 36,
         1, 2, 3, 4, 6,
         6, 9, 12, 18, 36,
         3, 4, 6, 9, 12]
DFF = 144
LN_EPS = 1e-6
BN_EPS = 1e-5
CH = 300           # token chunk within a half
MW = 120           # key-chunk rows
TH = 2             # 300-token q-chunks per half
VSTR = 72          # fixed per-m-chunk stride in vti
CONV_W = 302
TW = 300

# (slot, half) -> heads; program-level, core-independent
SH = []
for _s in range(NSLOT):
    SH.append((_s, 0, H0[_s]))
    if _s < len(H1):
        SH.append((_s, 1, H1[_s]))
ESH = {(s, j): e for e, (s, j, _h) in enumerate(SH)}
NSH = len(SH)  # 16


def _group_layout(h):
    """For h heads of dk=36//h: (dk, [(rows, [(strip, head_idx), ...]), ...])."""
    dk = D // h
    if dk > 32:
        return dk, [(36, [(0, 0)])]
    groups = []
    for g0 in range(0, h, 4):
        heads = list(range(g0, min(g0 + 4, h)))
        groups.append((32 * len(heads), [(32 * j, i) for j, i in enumerate(heads)]))
    return dk, groups


_QKOFF = {}
_QKTOT = 0
_VOFF = {}
_VTOT = 0
_GGOFF = {}
_GGTOT = 0
_EXKEY = {}
for _s, _j, _h in SH:
    _dk, _groups = _group_layout(_h)
    _QKOFF[(_s, _j)] = _QKTOT
    _QKTOT += sum(r for r, _ in _groups)
    _VOFF[(_s, _j)] = _VTOT
    _VTOT += D + _h + ((D + _h) % 2)
    _GGOFF[(_s, _j)] = _GGTOT
    _GGTOT += len(_groups)
    for _rows, _strips in _groups:
        _k = (_dk, len(_strips))
        if _k not in _EXKEY:
            _EXKEY[_k] = len(_EXKEY)
_NEXK = len(_EXKEY)


def _sin_pe():
    pos = np.arange(L, dtype=np.float32)[:, None]
    i = np.arange(0, D, 2, dtype=np.float32)
    div = np.exp(-(np.log(10000.0) / D) * i)
    pe = np.zeros((L, D), dtype=np.float32)
    pe[:, 0::2] = np.sin(pos * div)
    pe[:, 1::2] = np.cos(pos * div)
    return pe.T.copy()  # [36, 600]


def _chain_layer(grp, s, j):
    """Layer index for (slot, half) on cores of group grp, or None."""
    if grp == 0:
        if j == 0:
            return s                      # stack0: layers 0..8
        return 9 + s if s < 5 else None   # s1: layers 9..13 at slots 0..4
    else:
        if j == 0:
            return 14 + (s - 4) if s >= 4 else None  # s2 at slots 4..8
        return 19 + (s - 2) if 2 <= s < 7 else None  # s3 at slots 2..6


def _split_multi_waits(nc):
    """This container's walrus encodes at most ONE sem-wait per instruction."""
    import concourse.mybir as mybir

    n = 0
    for f in nc.m.functions:
        for bb in f.blocks:
            il = bb.instructions
            i = 0
            while i < len(il):
                inst = il[i]
                si = inst.sync_info
                if si is not None and si.on_wait and len(si.on_wait) > 1:
                    waits = list(si.on_wait)
                    for w in waits[:-1]:
                        ev = mybir.InstEventSemaphore(
                            name=f"I-wsplit-{n}",
                            engine=inst.engine,
                            ins=[], outs=[],
                            sync_info=mybir.SyncInfo(on_wait=[w], on_update=[]),
                        )
                        n += 1
                        il.insert(i, ev)
                        i += 1
                    inst.sync_info = mybir.SyncInfo(on_wait=[waits[-1]],
                                                    on_update=list(si.on_update or []))
                i += 1
    return n


def _build_nc():
    import concourse.bass as bass
    import concourse.mybir as mybir
    from concourse.tile import TileContext

    f32 = mybir.dt.float32
    f32r = mybir.dt.float32r
    AF = mybir.ActivationFunctionType
    OP = mybir.AluOpType

    nc = bass.Bass()
    dp = lambda name, shape, dt_=f32, isOutput=False: nc.declare_dram_parameter(name, shape, dt_, isOutput)
    xin_d = dp("xin", [D, T])
    wq_d = dp("wq", [D + 1, _QKTOT])   # aligned cols; row 36 = bias (q pre-scaled)
    wk_d = dp("wk", [D + 1, _QKTOT])
    wv_d = dp("wv", [D + 1, _VTOT])    # [1|V]-interleaved; row36: 1.0 at ones-col, bv at V cols
    wo_d = dp("wo", [128, _GGTOT * D])
    ex_d = dp("ex", [4, _NEXK * 128])
    lnA_d = dp("lnA", [1, 2 * NSH * D])      # 36*g per (e,ln)
    lnB_d = dp("lnB", [2, 2 * NSH * D])  # rows {-g, b}
    bo_d = dp("bo", [D, NSH])
    w1_d = dp("w1", [D, NSH * DFF])
    b1_d = dp("b1", [72, 2 * NSH])           # col 2e+fold
    w2_d = dp("w2", [73, NSH * 2 * D])  # row 72: b2 on fold0, 0 on fold1
    out_d = dp("zout", [D, T], f32, True)

    with TileContext(nc) as tc:
        with tc.tile_pool(name="const", bufs=1) as cpool, \
             tc.tile_pool(name="state", bufs=1) as spool, \
             tc.tile_pool(name="work", bufs=2) as wpool, \
             tc.tile_pool(name="attn", bufs=1) as apool, \
             tc.tile_pool(name="ps1", bufs=2, space="PSUM") as p1, \
             tc.tile_pool(name="ps2", bufs=2, space="PSUM") as p2, \
             tc.tile_pool(name="ps3", bufs=2, space="PSUM") as p3:

            def load(dram, shape):
                t = cpool.tile(shape, dram.dtype, tag=dram.name, name=dram.name)
                nc.sync.dma_start(out=t[:], in_=dram[:])
                return t

            wq = load(wq_d, [D + 1, _QKTOT])
            wk = load(wk_d, [D + 1, _QKTOT])
            wv = load(wv_d, [D + 1, _VTOT])
            wo = load(wo_d, [128, _GGTOT * D])
            ex = load(ex_d, [4, _NEXK * 128])
            lnA = load(lnA_d, [1, 2 * NSH * D])
            lnB = load(lnB_d, [2, 2 * NSH * D])
            bo = load(bo_d, [D, NSH])
            w1 = load(w1_d, [D, NSH * DFF])
            b1 = load(b1_d, [72, 2 * NSH])
            w2 = load(w2_d, [73, NSH * 2 * D])

            epsb = cpool.tile([1, 1], f32, tag="epsb")
            nc.vector.memset(epsb[:], float(D * D * LN_EPS))
            # stats mask columns: col0 = z rows (0:36), col1 = x^2 rows (64:100)
            ones2 = cpool.tile([100, 2], f32, tag="ones2")
            nc.vector.memset(ones2[:], 0.0)
            nc.vector.memset(ones2[0:D, 0:1], 1.0)
            nc.vector.memset(ones2[64:100, 1:2], 1.0)

            # residual rows 0:36, zero pad 36:64, x^2 scratch rows 64:100
            zz = spool.tile([100, T], f32, tag="zz")
            nc.vector.memset(zz[32:64, :], 0.0)
            nc.sync.dma_start(out=zz[0:D, :], in_=xin_d[:])
            y37a = spool.tile([D + 1, T], f32, tag="y37a")
            y37f = spool.tile([D + 1, T], f32, tag="y37f")
            nc.vector.memset(y37a[:], 1.0)
            nc.vector.memset(y37f[:], 1.0)
            # row0 = rstd'*s1 (per LN), row1 = const ones; one tile per (ln, half)
            st2s = {}
            for _k in range(4):
                _t = spool.tile([2, L], f32, tag=f"st2_{_k}", name=f"st2_{_k}")
                nc.vector.memset(_t[:], 1.0)
                st2s[_k] = _t
            ff = spool.tile([73, 2 * L], f32, tag="ff")
            nc.vector.memset(ff[:], 1.0)

            def layernorm(eln, toff, ydst, st2):
                """ydst[0:36, toff:toff+600] = LN(zz[0:36, toff:..]) with params lnA/lnB[eln]."""
                # x^2 on gpsimd
                nc.gpsimd.tensor_tensor(out=zz[64:100, toff:toff + L],
                                        in0=zz[0:D, toff:toff + L],
                                        in1=zz[0:D, toff:toff + L], op=OP.mult)
                s1b = wpool.tile([1, L], f32, tag="s1b", name="s1b")
                vsb = wpool.tile([1, L], f32, tag="vsb", name="vsb")
                qsb = wpool.tile([1, L], f32, tag="qsb", name="qsb")
                for c in range(2):
                    cs = slice(c * CH, (c + 1) * CH)
                    zsl = zz[:, toff + c * CH: toff + (c + 1) * CH]
                    ps1 = p1.tile([128, 512], f32, tag="ps", name="lns1")
                    ps2 = p1.tile([128, 512], f32, tag="ps", name="lns2")
                    nc.tensor.matmul(out=ps1[0:1, 0:CH], lhsT=ones2[:, 0:1], rhs=zsl)
                    nc.tensor.matmul(out=ps2[0:1, 0:CH], lhsT=ones2[:, 1:2], rhs=zsl)
                    nc.vector.tensor_copy(out=s1b[0:1, cs], in_=ps1[0:1, 0:CH])
                    nc.vector.tensor_tensor(out=qsb[0:1, cs], in0=s1b[0:1, cs],
                                            in1=s1b[0:1, cs], op=OP.mult)
                    nc.vector.scalar_tensor_tensor(out=vsb[0:1, cs], in0=ps2[0:1, 0:CH],
                                                   scalar=float(D), in1=qsb[0:1, cs],
                                                   op0=OP.mult, op1=OP.subtract)
                # rstd' = 1/sqrt(t + eps'): table sqrt + exact DVE reciprocal
                lvs = wpool.tile([1, L], f32, tag="lvs", name="lvs")
                nc.scalar.activation(out=lvs[:], in_=vsb[:], func=AF.Sqrt,
                                     bias=epsb[0:1, 0:1])
                rp = wpool.tile([1, L], f32, tag="rp", name="rp")
                nc.vector.reciprocal(out=rp[:], in_=lvs[:])
                nc.vector.tensor_tensor(out=st2[0:1, :], in0=rp[0:1, :],
                                        in1=s1b[0:1, :], op=OP.mult)
                for c in range(2):
                    pa_ = p1.tile([128, 512], f32, tag="ps", name="lnA")
                    pb_ = p1.tile([128, 512], f32, tag="ps", name="lnB")
                    nc.tensor.matmul(out=pa_[0:D, 0:CH],
                                     lhsT=lnA[0:1, eln * D:(eln + 1) * D],
                                     rhs=rp[0:1, c * CH:(c + 1) * CH],
                                     tile_position=(0, 0))
                    nc.tensor.matmul(out=pb_[0:D, 0:CH],
                                     lhsT=lnB[0:2, eln * D:(eln + 1) * D],
                                     rhs=st2[0:2, c * CH:(c + 1) * CH],
                                     tile_position=(0, 0))
                    ysl = ydst[0:D, toff + c * CH: toff + (c + 1) * CH]
                    nc.vector.tensor_tensor(out=ysl, in0=zz[0:D, toff + c * CH: toff + (c + 1) * CH],
                                            in1=pa_[0:D, 0:CH], op=OP.mult)
                    nc.vector.tensor_tensor(out=ysl, in0=ysl, in1=pb_[0:D, 0:CH], op=OP.add)

            for s, j, h in SH:
                e = ESH[(s, j)]
                dk, groups = _group_layout(h)
                krows = 36 if dk > 32 else 32
                toff = j * L
                vw = D + h
                vwe = vw + (vw % 2)
                voff = _VOFF[(s, j)]
                layernorm(2 * e, toff, y37a, st2s[j])

                # V^T + bias + ones via y37a as stationary
                vti = wpool.tile([MW, 5 * VSTR], f32, tag="vti", name="vti")
                for c in range(5):
                    pv = p1.tile([128, 512], f32, tag="ps", name="vtp")
                    nc.tensor.matmul(out=pv[0:MW, 0:vwe],
                                     lhsT=y37a[:, toff + c * MW: toff + (c + 1) * MW],
                                     rhs=wv[:, voff:voff + vwe],
                                     tile_position=(0, 0))
                    nc.vector.tensor_copy(out=vti[:, c * VSTR:c * VSTR + vw],
                                          in_=pv[0:MW, 0:vw])

                col = _QKOFF[(s, j)]
                for gl, (rows, strips) in enumerate(groups):
                    nh = len(strips)
                    gh = rows + 1 if h == 1 else rows
                    gg = _GGOFF[(s, j)] + gl
                    # Q/K projection (bias folded in row 36)
                    sq = p2.tile([128, 2, 512], f32, tag="sc", name="sq")
                    sk = p2.tile([128, 2, 512], f32, tag="sc", name="sk")
                    for c in range(2):
                        nc.tensor.matmul(out=sq[0:rows, c, 0:CH],
                                         lhsT=wq[:, col:col + rows],
                                         rhs=y37a[:, toff + c * CH: toff + (c + 1) * CH])
                        nc.tensor.matmul(out=sk[0:rows, c, 0:CH],
                                         lhsT=wk[:, col:col + rows],
                                         rhs=y37a[:, toff + c * CH: toff + (c + 1) * CH])
                    col += rows
                    qg = wpool.tile([128, L], f32, tag="qal", name="qg")
                    kg = wpool.tile([128, L], f32, tag="kal", name="kg")
                    nc.vector.tensor_copy(out=qg[0:rows, :], in_=sq[0:rows, :, 0:CH])
                    nc.vector.tensor_copy(out=kg[0:rows, :], in_=sk[0:rows, :, 0:CH])
                    # per-head c-pipeline: scores(c+1) overlaps exp/AV(c)
                    onorm = apool.tile([128, L], f32, tag="onorm", name="onorm",
                                       bufs=2)
                    zg = wpool.tile([4, L], f32, tag="zg", name="zg", bufs=1)
                    for hx, (strip, i) in enumerate(strips):
                        obase = strip if dk <= 31 else 0
                        poa_th = [p3.tile([128, 512], f32, tag="poa", name="poa")
                                  for _ in range(TH)]
                        for c in range(5):
                            sc_ = p2.tile([128, 2, 512], f32, tag="sc", name="sc")
                            for th in range(TH):
                                nc.tensor.matmul(
                                    out=sc_[0:MW, th, 0:CH],
                                    lhsT=kg[strip:strip + krows, c * MW:(c + 1) * MW],
                                    rhs=qg[strip:strip + krows, th * CH:(th + 1) * CH],
                                    tile_position=(strip, 0))
                            et = apool.tile([MW, 2 * CH], f32, tag=f"e{strip}",
                                            name=f"e{strip}", bufs=2)
                            nc.scalar.activation(out=et[:],
                                                 in_=sc_[0:MW, :, 0:CH], func=AF.Exp)
                            for th in range(TH):
                                nc.tensor.matmul(
                                    out=poa_th[th][0:dk + 1, 0:CH],
                                    lhsT=vti[:, c * VSTR + i * (dk + 1):
                                               c * VSTR + (i + 1) * (dk + 1)],
                                    rhs=et[:, th * CH:(th + 1) * CH],
                                    start=(c == 0), stop=(c == 4),
                                    tile_position=(0, 0))
                        for th in range(TH):
                            osl = onorm[obase:obase + dk + 1,
                                        th * CH:(th + 1) * CH]
                            if (hx + th) % 2 == 0:
                                nc.vector.tensor_copy(out=osl,
                                                      in_=poa_th[th][0:dk + 1, 0:CH])
                            else:
                                nc.scalar.activation(out=osl,
                                                     in_=poa_th[th][0:dk + 1, 0:CH],
                                                     func=AF.Copy)
                    for th in range(TH):
                        if dk <= 31:
                            nc.sync.dma_start(
                                out=zg[0:nh, th * CH:(th + 1) * CH],
                                in_=onorm[0:32 * nh:32, th * CH:(th + 1) * CH])
                        else:
                            nc.sync.dma_start(
                                out=zg[0:1, th * CH:(th + 1) * CH],
                                in_=onorm[0:1, th * CH:(th + 1) * CH])
                    rz = wpool.tile([4, L], f32, tag="rz", name="rz", bufs=1)
                    nc.vector.reciprocal(out=rz[0:nh, :], in_=zg[0:nh, :])
                    for th in range(TH):
                        pd = p1.tile([128, 512], f32, tag="ps", name="pd")
                        ek = _EXKEY[(dk, nh)]
                        nc.tensor.matmul(out=pd[0:gh, 0:CH],
                                         lhsT=ex[0:nh, ek * 128: ek * 128 + gh],
                                         rhs=rz[0:nh, th * CH:(th + 1) * CH],
                                         tile_position=(0, 0))
                        osl = onorm[0:gh, th * CH:(th + 1) * CH]
                        nc.vector.tensor_tensor(out=osl, in0=osl, in1=pd[0:gh, 0:CH],
                                                op=OP.mult)
                    for c in range(2):
                        po = p1.tile([128, 512], f32, tag="ps", name="po")
                        nc.tensor.matmul(out=po[0:D, 0:CH],
                                         lhsT=wo[0:gh, gg * D:(gg + 1) * D],
                                         rhs=onorm[0:gh, c * CH:(c + 1) * CH],
                                         tile_position=(0, 0))
                        zsl = zz[0:D, toff + c * CH: toff + (c + 1) * CH]
                        nc.vector.tensor_tensor(out=zsl, in0=zsl, in1=po[0:D, 0:CH],
                                                op=OP.add)
                zsl = zz[0:D, toff:toff + L]
                nc.vector.tensor_scalar(out=zsl, in0=zsl, scalar1=bo[:, e:e + 1],
                                        scalar2=None, op0=OP.add)

                # FFN
                layernorm(2 * e + 1, toff, y37f, st2s[2 + j])
                for fold in range(2):
                    sf = p2.tile([128, 2, 512], f32, tag="sc", name="sf")
                    for c in range(2):
                        nc.tensor.matmul(
                            out=sf[0:72, c, 0:CH],
                            lhsT=w1[:, e * DFF + fold * 72: e * DFF + (fold + 1) * 72],
                            rhs=y37f[0:D, toff + c * CH: toff + (c + 1) * CH])
                    nc.scalar.activation(out=ff[0:72, fold * L:(fold + 1) * L],
                                         in_=sf[0:72, :, 0:CH], func=AF.Gelu_apprx_tanh,
                                         bias=b1[:, 2 * e + fold:2 * e + fold + 1])
                for c in range(2):
                    so = p1.tile([128, 512], f32, tag="ps", name="so")
                    for fold in range(2):
                        nc.tensor.matmul(
                            out=so[0:D, 0:CH],
                            lhsT=w2[:, (2 * e + fold) * D:(2 * e + fold + 1) * D],
                            rhs=ff[:, fold * L + c * CH: fold * L + (c + 1) * CH],
                            start=(fold == 0), stop=(fold == 1))
                    zsl = zz[0:D, toff + c * CH: toff + (c + 1) * CH]
                    nc.vector.tensor_tensor(out=zsl, in0=zsl, in1=so[0:D, 0:CH],
                                            op=OP.add)

            # SE gating per half, write out
            gated = spool.tile([D, T], f32, tag="gated")
            gsc = wpool.tile([D, 4], f32, tag="gsc")
            for j in range(2):
                toff = j * L
                nc.vector.tensor_reduce(out=gsc[:, j:j + 1], in_=zz[0:D, toff:toff + L],
                                        axis=mybir.AxisListType.X, op=OP.add)
                nc.scalar.activation(out=gsc[:, 2 + j:3 + j], in_=gsc[:, j:j + 1],
                                     func=AF.Sigmoid, scale=1.0 / L)
                nc.vector.tensor_scalar(out=gated[:, toff:toff + L],
                                        in0=zz[0:D, toff:toff + L],
                                        scalar1=gsc[:, 2 + j:3 + j],
                                        scalar2=None, op0=OP.mult)
            nc.sync.dma_start(out=out_d[:], in_=gated[:])
    return nc


def _build_nc2():
    """Launch 2: conv1d(144->36,k=3,pad=1) + BN + ReLU on a [144, 302] window."""
    import concourse.bass as bass
    import concourse.mybir as mybir
    from concourse.tile import TileContext

    f32 = mybir.dt.float32
    f32r = mybir.dt.float32r
    AF = mybir.ActivationFunctionType
    XW = 2 * CONV_W
    WW = 2 * 3 * D

    nc = bass.Bass()
    xfw_d = nc.declare_dram_parameter("xfw", [72, XW + WW], f32, False)
    sb_d = nc.declare_dram_parameter("sb2", [D, 2], f32, False)
    out_d = nc.declare_dram_parameter("yout", [D, TW], f32, True)

    with TileContext(nc) as tc:
        with tc.tile_pool(name="sb", bufs=1) as sb, \
             tc.tile_pool(name="ps", bufs=2, space="PSUM") as ps:
            xfw = sb.tile([72, XW + WW], f32, tag="xfw")
            nc.sync.dma_start(out=xfw[:], in_=xfw_d[:])
            sb2 = sb.tile([D, 2], f32, tag="sb2")
            nc.sync.dma_start(out=sb2[:], in_=sb_d[:])

            pso = ps.tile([D, TW], f32, tag="pso")
            first = True
            for k in range(3):
                for half in range(2):
                    nc.tensor.matmul(
                        out=pso[:],
                        lhsT=xfw[:, XW + half * 3 * D + k * D:
                                 XW + half * 3 * D + (k + 1) * D],
                        rhs=xfw[:, half * CONV_W + k: half * CONV_W + k + TW],
                        start=first, stop=(k == 2 and half == 1))
                    first = False
            yo = sb.tile([D, TW], f32, tag="yo")
            nc.scalar.activation(out=yo[:], in_=pso[:], func=AF.Relu,
                                 scale=sb2[:, 0:1], bias=sb2[:, 1:2])
            nc.sync.dma_start(out=out_d[:], in_=yo[:])
    return nc


_CACHE = {}
LAST_RESULTS = []


def _pack_core_weights(grp, Wq, bq, Wk, bk, Wv, bv, Wo, bo,
                       ln1_g, ln1_b, ln2_g, ln2_b, W1, b1, W2, b2):
    wq_p = np.zeros((D + 1, _QKTOT), np.float32)
    wk_p = np.zeros((D + 1, _QKTOT), np.float32)
    wv_p = np.zeros((D + 1, _VTOT), np.float32)
    wo_p = np.zeros((128, _GGTOT * D), np.float32)
    ex_p = np.zeros((4, _NEXK * 128), np.float32)
    lnA_p = np.zeros((1, 2 * NSH * D), np.float32)
    lnB_p = np.zeros((2, 2 * NSH * D), np.float32)
    bo_p = np.zeros((D, NSH), np.float32)
    w1_p = np.zeros((D, NSH * DFF), np.float32)
    b1_p = np.zeros((72, 2 * NSH), np.float32)
    w2_p = np.zeros((73, NSH * 2 * D), np.float32)

    for s, j, h in SH:
        e = ESH[(s, j)]
        dk, groups = _group_layout(h)
        li = _chain_layer(grp, s, j)
        # ones-cols of interleaved V are always 1.0 (Z row); exal mask always set
        voff = _VOFF[(s, j)]
        for i in range(h):
            wv_p[D, voff + i * (dk + 1)] = 1.0
        for gl, (rows, strips) in enumerate(groups):
            ek = _EXKEY[(dk, len(strips))]
            for jj, (strip, i) in enumerate(strips):
                base = 1 if h == 1 else strip + 1
                ex_p[jj, ek * 128 + base: ek * 128 + base + dk] = 1.0
        # LN params (identity: g=1, b=0)
        for ln in range(2):
            eln = 2 * e + ln
            if li is not None:
                g = (ln1_g if ln == 0 else ln2_g)[li]
                b = (ln1_b if ln == 0 else ln2_b)[li]
            else:
                g = np.ones(D, np.float32)
                b = np.zeros(D, np.float32)
            lnA_p[0, eln * D:(eln + 1) * D] = D * g
            lnB_p[0, eln * D:(eln + 1) * D] = -g
            lnB_p[1, eln * D:(eln + 1) * D] = b
        if li is None:
            continue
        sc = 1.0 / np.sqrt(dk)
        col = _QKOFF[(s, j)]
        for rows, strips in groups:
            for strip, i in strips:
                wq_p[0:D, col + strip: col + strip + dk] = Wq[li][:, i * dk:(i + 1) * dk] * sc
                wq_p[D, col + strip: col + strip + dk] = bq[li][i * dk:(i + 1) * dk] * sc
                wk_p[0:D, col + strip: col + strip + dk] = Wk[li][:, i * dk:(i + 1) * dk]
                wk_p[D, col + strip: col + strip + dk] = bk[li][i * dk:(i + 1) * dk]
            col += rows
        for i in range(h):
            c0 = voff + i * (dk + 1) + 1
            wv_p[0:D, c0:c0 + dk] = Wv[li][:, i * dk:(i + 1) * dk]
            wv_p[D, c0:c0 + dk] = bv[li][i * dk:(i + 1) * dk]
        for gl, (rows, strips) in enumerate(groups):
            gg = _GGOFF[(s, j)] + gl
            for strip, i in strips:
                base = 1 if h == 1 else strip + 1
                wo_p[base:base + dk, gg * D:(gg + 1) * D] = Wo[li][i * dk:(i + 1) * dk, :]
        bo_p[:, e] = bo[li]
        w1_p[:, e * DFF:(e + 1) * DFF] = W1[li]
        b1_p[:, 2 * e] = b1[li][:72]
        b1_p[:, 2 * e + 1] = b1[li][72:]
        w2_p[0:72, 2 * e * D:(2 * e + 1) * D] = W2[li][:72]
        w2_p[0:72, (2 * e + 1) * D:(2 * e + 2) * D] = W2[li][72:]
        w2_p[72, 2 * e * D:(2 * e + 1) * D] = b2[li]
    return dict(wq=wq_p, wk=wk_p, wv=wv_p, wo=wo_p, ex=ex_p, lnA=lnA_p,
                lnB=lnB_p, bo=bo_p, w1=w1_p, b1=b1_p, w2=w2_p)


def kernel(x, ln1_g, ln1_b, Wq, bq, Wk, bk, Wv, bv, Wo, bo,
           ln2_g, ln2_b, W1, b1, W2, b2,
           conv_w, conv_b, bn_g, bn_b, bn_mean, bn_var):
    from concourse.bass_utils import run_bass_kernel_spmd

    args = [np.asarray(a, np.float32) for a in
            (x, ln1_g, ln1_b, Wq, bq, Wk, bk, Wv, bv, Wo, bo,
             ln2_g, ln2_b, W1, b1, W2, b2)]
    (x, ln1_g, ln1_b, Wq, bq, Wk, bk, Wv, bv, Wo, bo,
     ln2_g, ln2_b, W1, b1, W2, b2) = args
    conv_w = np.asarray(conv_w, np.float32)
    conv_b = np.asarray(conv_b, np.float32)
    bn_g = np.asarray(bn_g, np.float32); bn_b = np.asarray(bn_b, np.float32)
    bn_mean = np.asarray(bn_mean, np.float32); bn_var = np.asarray(bn_var, np.float32)

    if "nc1" not in _CACHE:
        _CACHE["nc1"] = _build_nc()
        _split_multi_waits(_CACHE["nc1"])
        _CACHE["nc2"] = _build_nc2()
        _split_multi_waits(_CACHE["nc2"])
    nc1, nc2 = _CACHE["nc1"], _CACHE["nc2"]

    pe = _sin_pe()
    packed = {}
    in_maps = []
    for c in range(8):
        b, grp = c % 4, c // 4
        if grp not in packed:
            packed[grp] = _pack_core_weights(grp, Wq, bq, Wk, bk, Wv, bv, Wo, bo,
                                             ln1_g, ln1_b, ln2_g, ln2_b, W1, b1, W2, b2)
        xin = np.tile(x[b] + pe, (1, 2)).astype(np.float32)
        in_maps.append(dict(xin=xin, **packed[grp]))
    LAST_RESULTS.clear()
    r1 = run_bass_kernel_spmd(nc1, in_maps, list(range(8)))
    LAST_RESULTS.append(r1)
    res1 = r1.results

    # zout halves -> branches: core c (b=c%4, grp=c//4): half j -> branch 2*grp+j
    xf = np.zeros((B, DFF, L), np.float32)
    for c in range(8):
        b, grp = c % 4, c // 4
        zo = res1[c]["zout"]
        xf[b, (2 * grp) * D:(2 * grp + 1) * D] = zo[:, :L]
        xf[b, (2 * grp + 1) * D:(2 * grp + 2) * D] = zo[:, L:]

    scale = bn_g / np.sqrt(bn_var + BN_EPS)
    bias = bn_b + scale * (conv_b - bn_mean)
    wc = np.zeros((DFF, 3 * D), np.float32)
    for k in range(3):
        wc[:, k * D:(k + 1) * D] = conv_w[:, :, k].T
    wc2 = wc.reshape(2, 72, 3 * D).transpose(1, 0, 2).reshape(72, 2 * 3 * D).copy()
    in_maps2 = []
    for c in range(8):
        b, half = c // 2, c % 2
        win = np.zeros((DFF, CONV_W), np.float32)
        lo = half * TW - 1
        s0 = max(lo, 0)
        s1 = min(lo + CONV_W, L)
        win[:, s0 - lo: s1 - lo] = xf[b][:, s0:s1]
        win2 = win.reshape(2, 72, CONV_W).transpose(1, 0, 2).reshape(72, 2 * CONV_W)
        xfw = np.concatenate([win2, wc2], axis=1).astype(np.float32)
        sb2 = np.stack([scale, bias], axis=1).astype(np.float32)
        in_maps2.append(dict(xfw=xfw.copy(), sb2=sb2.copy()))
    r2 = run_bass_kernel_spmd(nc2, in_maps2, list(range(8)))
    LAST_RESULTS.append(r2)
    res2 = r2.results

    out = np.zeros((B, D, L), np.float32)
    for c in range(8):
        b, half = c // 2, c % 2
        out[b][:, half * TW:(half + 1) * TW] = res2[c]["yout"]
    return out
